# revision 1
# baseline (speedup 1.0000x reference)
"""Kronecker product kernel for Trainium2 (Bass/Tile), 8-core SPMD.

out[i*64+p, j*64+q] = A[i, j] * B[p, q] with A: (128, 128) f32, B: (64, 64) f32.
Output: (8192, 8192) f32 (256 MB) — memory-regime, output-write bound.

Sharding: A's row dim across 8 cores (16 rows each). Each core owns a
(1024, 8192) block-row of the output (32 MB) and holds a full replica of B.

Per-core layout: an output tile is [128 partitions, 8192] where the partition
dim covers 2 A-rows x 64 B-rows and the free dim is (j, q). Each tile is a
fully contiguous 4 MB DRAM write (128 rows x 32 KB), keeping store DMAs at
line rate.

A-value replication across partitions is done on the PE: a constant [2, 128]
selection matrix S (S[d, m] = 1 iff m // 64 == d) turns two A rows into a
[128, 128] PSUM tile ae[(d, p), j] = A[2t + d, j]. The DVE then computes
o[(d, p), (j, q)] = ae[(d, p), j] * b2[(d, p), q] with stride-0 (broadcast)
input access patterns. This avoids broadcast-source DMAs, which the CoreSim
race tracker mishandles.
"""

import numpy as np

import concourse.bacc as bacc
import concourse.bass as bass
import concourse.mybir as mybir
from concourse.bass_utils import run_bass_kernel_spmd
from concourse.tile import TileContext

N_CORES = 8
AR, AC = 128, 128
BR, BC = 64, 64
ROWS_PER_CORE = AR // N_CORES        # 16 A-rows per core
OUT_ROWS = ROWS_PER_CORE * BR        # 1024 output rows per core
OUT_COLS = AC * BC                   # 8192
I_PER_TILE = 128 // BR               # 2 A-rows fill the 128 partitions
N_TILES = ROWS_PER_CORE // I_PER_TILE  # 8 output tiles of [128, 8192] per core

_cache: dict = {}


def _build() -> bass.Bass:
    nc = bacc.Bacc(None)
    a = nc.dram_tensor(
        "a_shard", [ROWS_PER_CORE, AC], mybir.dt.float32, kind="ExternalInput"
    )
    b = nc.dram_tensor("b_full", [BR, BC], mybir.dt.float32, kind="ExternalInput")
    sel = nc.dram_tensor(
        "sel", [I_PER_TILE, 128], mybir.dt.float32, kind="ExternalInput"
    )
    out = nc.dram_tensor(
        "out_shard", [OUT_ROWS, OUT_COLS], mybir.dt.float32, kind="ExternalOutput"
    )

    with TileContext(nc) as tc:
        with (
            tc.tile_pool(name="consts", bufs=1) as consts,
            tc.tile_pool(name="psum", bufs=4, space="PSUM") as psum,
            tc.tile_pool(name="opool", bufs=3) as opool,
        ):
            # B replicated twice along partitions: b2[(d, p), q] = B[p, q]
            b2_raw = consts.tile([128, BC], mybir.dt.float32, tag="b2_raw")
            nc.sync.dma_start(out=b2_raw[:BR, :], in_=b[:, :])
            nc.sync.dma_start(out=b2_raw[BR:, :], in_=b[:, :])

            # A rows packed on 2 partitions: a2[d, t*128 + j] = A[2t + d, j]
            a2_raw = consts.tile(
                [I_PER_TILE, N_TILES * AC], mybir.dt.float32, tag="a2_raw"
            )
            nc.sync.dma_start(
                out=a2_raw[:].rearrange("d (t j) -> d t j", j=AC),
                in_=a.rearrange("(t d) j -> d t j", d=I_PER_TILE),
            )

            # Selection matrix (host-supplied): S[d, m] = 1 iff m // 64 == d
            s2_raw = consts.tile([I_PER_TILE, 128], mybir.dt.float32, tag="s2_raw")
            nc.sync.dma_start(out=s2_raw[:, :], in_=sel[:, :])

            # Funnel both PE operands through DVE copies so every matmul's
            # input deps collapse onto the single DVE semaphore lane — the
            # Matmult load-weights slot supports very few sync waits.
            a2 = consts.tile([I_PER_TILE, N_TILES * AC], mybir.dt.float32, tag="a2")
            nc.vector.tensor_copy(a2[:, :], a2_raw[:, :])
            s2 = consts.tile([I_PER_TILE, 128], mybir.dt.float32, tag="s2")
            nc.vector.tensor_copy(s2[:, :], s2_raw[:, :])
            b2 = consts.tile([128, BC], mybir.dt.float32, tag="b2")
            nc.vector.tensor_copy(b2[:BR, :], b2_raw[:BR, :])
            nc.vector.tensor_copy(b2[BR:, :], b2_raw[BR:, :])

            for t in range(N_TILES):
                # ae[(d, p), j] = A[2t + d, j] via PE broadcast
                ae = psum.tile([128, AC], mybir.dt.float32, tag="ae")
                nc.tensor.matmul(
                    ae[:, :],
                    s2[:, :],
                    a2[:, bass.ts(t, AC)],
                    start=True,
                    stop=True,
                )
                o = opool.tile([128, OUT_COLS], mybir.dt.float32, tag="o")
                nc.vector.tensor_tensor(
                    o[:].rearrange("m (j q) -> m j q", q=BC),
                    ae[:, :, None].to_broadcast([128, AC, BC]),
                    b2[:, None, :].to_broadcast([128, AC, BC]),
                    mybir.AluOpType.mult,
                )
                nc.sync.dma_start(out=out[bass.ts(t, 128), :], in_=o[:])
    nc.compile()
    return nc


def kernel(A: np.ndarray, B: np.ndarray) -> np.ndarray:
    A = np.ascontiguousarray(np.asarray(A, dtype=np.float32))
    B = np.ascontiguousarray(np.asarray(B, dtype=np.float32))
    assert A.shape == (AR, AC) and B.shape == (BR, BC)

    nc = _cache.get("nc")
    if nc is None:
        nc = _cache["nc"] = _build()

    sel = np.zeros((I_PER_TILE, 128), dtype=np.float32)
    for d in range(I_PER_TILE):
        sel[d, d * BR : (d + 1) * BR] = 1.0

    in_maps = [
        {
            "a_shard": A[c * ROWS_PER_CORE : (c + 1) * ROWS_PER_CORE],
            "b_full": B,
            "sel": sel,
        }
        for c in range(N_CORES)
    ]
    res = run_bass_kernel_spmd(nc, in_maps, core_ids=list(range(N_CORES)))
    return np.concatenate([r["out_shard"] for r in res.results], axis=0)


if __name__ == "__main__":
    rng = np.random.default_rng(0)
    A = rng.standard_normal((AR, AC), dtype=np.float32)
    B = rng.standard_normal((BR, BC), dtype=np.float32)
    got = kernel(A, B)
    want = np.kron(A, B)
    err = np.abs(got - want).max()
    print("max abs err:", err, "ref scale:", np.abs(want).max())

